# revision 14
# baseline (speedup 1.0000x reference)
"""NT-Xent loss on 8 Trainium2 NeuronCores.

Full inputs in, full (scalar) output out. Row-parallel sharding: core c
owns rows [1024c, 1024c+1024) of the 8192-row feature matrix; inputs are
row-rotated per core so the single SPMD program sees its own rows at
local positions 0..1023 (static diagonal mask / positive-pair columns).

v6 = v4 compute (numerics unchanged from the passing run) with the
engine queues descheduled:
  - bf16 features from host, normalized on-device into fp8(e4m3) x16
    (free via Exp bias); main matmul is DoubleRow fp8 (K=256 per MM).
  - chunk-0 norms replicated in halves over both HWDGE rings (shortest
    cold start); chunks 1-3 compact (full-width Ln = PSUM->SBUF move,
    Exp on a [128,16] DMA compaction, GPSIMD partition_broadcast back).
  - 9 of 32 exp+rowsum tiles run off-ACT mid-program: DVE Schraudolph
    bit-exp (int16 bits == bf16 exp), GPSIMD tree-adds, deferred DVE
    reduces.
  - emission order keeps every long op out of the path between a
    producer and the next tile's consumer: norm mul work is kg-split
    and interleaved between offload tiles, chunk chains pre-run under
    the previous group's exp stream.
  Host sums the 8 per-core [128,2] partials.
"""
import numpy as np
import ml_dtypes

import concourse.bass as bass  # noqa: F401
import concourse.tile as tile
import concourse.bacc as bacc_mod
from concourse import bacc, mybir
from concourse.bass_utils import run_bass_kernel_spmd
from concourse.hw_specs import get_activation_tables as _real_tables

B, D = 4096, 256
N = 2 * B                # 8192 rows/cols of sim matrix
NCORES = 8
RPC = N // NCORES        # 1024 rows per core
TEMP = 0.07
SCALE = 1.0 / TEMP
KG = 2                   # contraction groups: D = 256 = 2 * 128
CH = 2048                # column chunk (psum tile width)
NCH = N // CH            # 4 chunks
MT = RPC // 128          # 8 M-tiles per core
NEG = -1.0e9
LN16 = 2.772588722239781  # ln(16): fp8 features scaled x16, sim x256
SCH_A = 128.0 / 0.6931471805599453
SCH_B = 127.0 * 128.0 - 6.0
# tiles whose exp+rowsum run on DVE+GPSIMD instead of ACT (g>=1: no diag
# mask, |sim|<=1 keeps bits in int16 range; none late in g3 so the
# offload pipeline drains under remaining ACT tiles)
DVE_TILES = ({(1, m) for m in (1, 3, 5, 7)} | {(2, m) for m in (1, 3, 5, 7)}
             | {(3, 1)})

AF = mybir.ActivationFunctionType
ALU = mybir.AluOpType
AX = mybir.AxisListType
f32 = mybir.dt.float32
bf16 = mybir.dt.bfloat16
fp8 = mybir.dt.float8e4
i16 = mybir.dt.int16
DR = mybir.MatmulPerfMode.DoubleRow

_CACHE = {}


def _pinned_tables(arch):
    """Keep Exp/Ln only in natural_log_exp_and_others so the act-table
    insertion pass picks one set for the whole kernel (no reload thrash)."""
    tables = _real_tables(arch)
    out = {}
    for name, funcs in tables.items():
        if name != "natural_log_exp_and_others":
            funcs = {f for f in funcs if f.name not in ("Exp", "Ln")}
        out[name] = funcs
    return out


def _build_nc():
    bacc_mod.get_activation_tables = _pinned_tables
    nc = bacc.Bacc("TRN2", target_bir_lowering=False, debug=False,
                   enable_asserts=False, num_devices=NCORES,
                   num_swdge_queues=2)

    ztb_d = nc.dram_tensor("ztb", [KG, 128, N], bf16, kind="ExternalInput")
    cones_d = nc.dram_tensor("cones", [128, 128], bf16, kind="ExternalInput")
    negid_d = nc.dram_tensor("negid", [128, 128], f32, kind="ExternalInput")
    out_d = nc.dram_tensor("out", [128, 2], f32, kind="ExternalOutput")

    with tile.TileContext(nc) as tc:
        with (
            tc.tile_pool(name="singles", bufs=1) as singles,
            tc.tile_pool(name="nfp", bufs=1) as nfp,
            tc.tile_pool(name="sqp", bufs=2) as sqp,
            tc.tile_pool(name="invp", bufs=2) as invp,
            tc.tile_pool(name="expp", bufs=2) as expp,
            tc.tile_pool(name="ebp", bufs=3) as ebp,
            tc.tile_pool(name="t3p", bufs=6) as t3p,
            tc.tile_pool(name="ps", bufs=2, space="PSUM") as ps,
        ):
            cones = singles.tile([128, 128], bf16, tag="cones")
            nc.gpsimd.dma_start(out=cones, in_=cones_d.ap())
            negid = singles.tile([128, 128], f32, tag="negid")
            nc.gpsimd.dma_start(out=negid, in_=negid_d.ap())

            bias16 = singles.tile([128, 1], f32, tag="bias16")
            nc.gpsimd.memset(bias16, LN16)

            sums = singles.tile([128, MT * NCH], f32, tag="sums")
            nc.vector.memset(sums, 0.0)
            gsums = singles.tile([128, MT * NCH], f32, tag="gsums")
            nc.gpsimd.memset(gsums, 0.0)

            # warm the gpsimd ext-isa library (IRAM load ~6us) off the
            # critical path
            warm = singles.tile([128, 128], bf16, tag="warm")
            nc.gpsimd.partition_broadcast(warm, cones[0:1, :])

            ztb = [nfp.tile([128, KG, CH], bf16, tag=f"ztb{g}",
                            name=f"ztb{g}") for g in range(NCH)]
            nf = [nfp.tile([128, KG, CH], fp8, tag=f"nf{g}",
                           name=f"nf{g}") for g in range(NCH)]

            ztb_ap = ztb_d.ap()

            def load(g, col0, width, eng):
                eng.dma_start(
                    out=ztb[g][:, :, col0:col0 + width],
                    in_=ztb_ap[:, :, CH * g + col0:CH * g + col0 + width]
                    .rearrange("k p c -> p k c"))

            load(0, 0, 1024, nc.sync)      # chunk 0 split over both HWDGE
            load(0, 1024, 1024, nc.scalar)  # rings for earliest start
            load(1, 0, CH, nc.scalar)
            load(2, 0, CH, nc.gpsimd)
            load(3, 0, CH, nc.sync)

            CMP = CH // 128   # compact width: 16

            def colsum(dst_ps, src, width):
                for n in range(width // 512):
                    for kg in range(KG):
                        nc.tensor.matmul(
                            dst_ps[:, 512 * n:512 * (n + 1)], cones,
                            src[:, kg, 512 * n:512 * (n + 1)],
                            start=(kg == 0), stop=(kg == KG - 1))

            def norm_rep_half(h):
                """Chunk-0 half h: replicated inverse norms (no DMA round
                trip) -> nf[0] cols [1024h, 1024h+1024)."""
                c0, w = 1024 * h, 1024
                zs = ztb[0][:, :, c0:c0 + w]
                sq_t = sqp.tile([128, KG, w], bf16, tag="sq0",
                                name=f"sq0_{h}")
                nc.vector.tensor_mul(sq_t, zs, zs)
                nn_ps = ps.tile([128, w], f32, tag="ps", name=f"nn0_{h}")
                colsum(nn_ps, sq_t, w)
                lnv = invp.tile([128, w], f32, tag="lnv", name=f"lnv0_{h}")
                nc.scalar.activation(lnv, nn_ps, AF.Ln)
                invrep = invp.tile([128, w], bf16, tag="invrep",
                                   name=f"invrep0_{h}")
                nc.scalar.activation(invrep, lnv, AF.Exp, scale=-0.5,
                                     bias=bias16[:, 0:1])
                for kg in range(KG):
                    nc.vector.tensor_mul(nf[0][:, kg, c0:c0 + w],
                                         zs[:, kg, :], invrep)

            invreps = {}

            def norm_pre(g):
                """Chunk g: everything up to the replicated inverse norms
                (sq -> colsum -> Ln full width -> compact Exp -> re-expand
                via GPSIMD). No DVE op longer than a square."""
                sq_t = sqp.tile([128, KG, CH], bf16, tag="sq", name=f"sq{g}")
                nc.vector.tensor_mul(sq_t, ztb[g], ztb[g])
                nn_ps = ps.tile([128, CH], f32, tag="ps", name=f"nn{g}")
                colsum(nn_ps, sq_t, CH)
                lnv = invp.tile([128, CH], f32, tag="lnv", name=f"lnv{g}")
                nc.scalar.activation(lnv, nn_ps, AF.Ln)
                lnc = invp.tile([128, CMP], f32, tag="lnc", name=f"lnc{g}")
                nc.sync.dma_start(out=lnc, in_=lnv[0:1, :])
                invc = invp.tile([128, CMP], bf16, tag="invc",
                                 name=f"invc{g}")
                nc.scalar.activation(invc, lnc, AF.Exp, scale=-0.5,
                                     bias=bias16[:, 0:1])
                invrow = invp.tile([1, CH], bf16, tag="invrow",
                                   name=f"invrow{g}")
                nc.sync.dma_start(out=invrow, in_=invc)
                invrep = invp.tile([128, CH], bf16, tag="invrep",
                                   name=f"invrep{g}")
                nc.gpsimd.partition_broadcast(invrep, invrow)
                invreps[g] = invrep

            def norm_mul(g, kg):
                nc.vector.tensor_mul(nf[g][:, kg, :], ztb[g][:, kg, :],
                                     invreps[g])

            pending_red = []

            def flush_red(keep=0):
                while len(pending_red) > keep:
                    t3, idx = pending_red.pop(0)
                    nc.vector.tensor_reduce(gsums[:, idx:idx + 1], t3,
                                            axis=AX.X, op=ALU.add)

            def main_tile(g, m):
                sim_ps = ps.tile([128, CH], f32, tag="ps", name="sim_ps")
                lhsT = nf[0][:, :, 128 * m:128 * (m + 1)]
                for n in range(CH // 512):
                    nc.tensor.matmul(
                        sim_ps[:, 512 * n:512 * (n + 1)], lhsT,
                        nf[g][:, :, 512 * n:512 * (n + 1)],
                        start=True, stop=True, perf_mode=DR)
                idx = m * NCH + g
                if g == 0:
                    sl = sim_ps[:, 128 * m:128 * (m + 1)]
                    nc.vector.tensor_add(sl, sl, negid)
                if (g, m) in DVE_TILES:
                    eb = ebp.tile([128, CH], i16, tag="eb", name="eb")
                    nc.vector.tensor_scalar(
                        out=eb, in0=sim_ps,
                        scalar1=SCH_A * SCALE / 256.0, scalar2=SCH_B,
                        op0=ALU.mult, op1=ALU.add)
                    ebf = eb[:, :].bitcast(bf16)
                    t1 = ebp.tile([128, 1024], bf16, tag="t1", name="t1")
                    nc.gpsimd.tensor_add(t1, ebf[:, 0:1024],
                                         ebf[:, 1024:2048])
                    t2 = ebp.tile([128, 512], bf16, tag="t2", name="t2")
                    nc.gpsimd.tensor_add(t2, t1[:, 0:512], t1[:, 512:1024])
                    t3 = t3p.tile([128, 256], bf16, tag="t3", name="t3")
                    nc.gpsimd.tensor_add(t3, t2[:, 0:256], t2[:, 256:512])
                    pending_red.append((t3, idx))
                    flush_red(keep=3)
                else:
                    exp_sc = expp.tile([128, CH], bf16, tag="exp",
                                       name="exp_sc")
                    nc.scalar.activation(exp_sc, sim_ps, AF.Exp,
                                         scale=SCALE / 256.0,
                                         accum_out=sums[:, idx:idx + 1])

            # ---- schedule ----
            norm_rep_half(0)
            norm_rep_half(1)
            norm_pre(1)                     # chain runs under g0's stream
            main_tile(0, 0)
            main_tile(0, 1)
            norm_mul(1, 0)
            main_tile(0, 2)
            main_tile(0, 3)
            norm_mul(1, 1)
            for m in range(4, MT):
                main_tile(0, m)

            norm_pre(2)                     # Ln2/Exp2c between e07 and e10
            norm_pre(3)
            main_tile(1, 0)
            main_tile(1, 1)
            norm_mul(2, 0)
            main_tile(1, 2)
            main_tile(1, 3)
            norm_mul(2, 1)
            main_tile(1, 4)
            main_tile(1, 5)
            norm_mul(3, 0)
            main_tile(1, 6)
            main_tile(1, 7)
            norm_mul(3, 1)

            # positive term: partner of local row i is local column i of
            # group 2. pos_i = <nf_i, nf_{i+4096}> / 256.
            fin = singles.tile([128, 2], f32, tag="fin")
            tmp_pos = sqp.tile([128, KG, RPC], bf16, tag="tpos")
            for kg in range(KG):
                nc.vector.tensor_mul(tmp_pos[:, kg, :],
                                     nf[0][:, kg, 0:RPC],
                                     nf[2][:, kg, 0:RPC])
            pos_ps = ps.tile([128, RPC], f32, tag="ps")
            colsum(pos_ps, tmp_pos, RPC)
            nc.vector.tensor_reduce(fin[:, 1:2], pos_ps, axis=AX.X,
                                    op=ALU.add)

            for m in range(MT):
                main_tile(2, m)
            for m in range(MT):
                main_tile(3, m)
            flush_red()

            # lse per row: ln(ACT sums + DVE sums)
            rowsum = singles.tile([128, MT], f32, tag="rowsum")
            nc.vector.tensor_reduce(
                rowsum, sums.rearrange("p (m g) -> p m g", g=NCH),
                axis=AX.X, op=ALU.add)
            growsum = singles.tile([128, MT], f32, tag="growsum")
            nc.vector.tensor_reduce(
                growsum, gsums.rearrange("p (m g) -> p m g", g=NCH),
                axis=AX.X, op=ALU.add)
            nc.vector.tensor_add(rowsum, rowsum, growsum)
            lse8 = singles.tile([128, MT], f32, tag="lse8")
            nc.scalar.activation(lse8, rowsum, AF.Ln)
            nc.vector.tensor_reduce(fin[:, 0:1], lse8, axis=AX.X, op=ALU.add)
            # host combines: sum_p fin[p,0] - SCALE/256 * fin[0,1]
            nc.sync.dma_start(out=out_d.ap(), in_=fin)

    nc.compile()
    return nc


def _get_nc():
    if "nc" not in _CACHE:
        _CACHE["nc"] = _build_nc()
    return _CACHE["nc"]


def _in_maps(z_i, z_j):
    feats = np.concatenate([np.asarray(z_i, dtype=np.float32),
                            np.asarray(z_j, dtype=np.float32)], axis=0)
    cones = np.ones((128, 128), dtype=ml_dtypes.bfloat16)
    negid = (NEG * np.eye(128)).astype(np.float32)
    maps = []
    for c in range(NCORES):
        zr = np.roll(feats, -RPC * c, axis=0)          # [N, D]
        ztb = np.ascontiguousarray(zr.T).reshape(KG, 128, N).astype(
            ml_dtypes.bfloat16)
        maps.append({"ztb": ztb, "cones": cones, "negid": negid})
    return maps


def kernel(z_i, z_j, _trace=False, _trace_kwargs=None):
    nc = _get_nc()
    maps = _in_maps(z_i, z_j)
    res = run_bass_kernel_spmd(nc, maps, core_ids=list(range(NCORES)),
                               trace=_trace, **(_trace_kwargs or {}))
    total = 0.0
    for c in range(NCORES):
        fin = np.asarray(res.results[c]["out"], dtype=np.float64)
        total += fin[:, 0].sum() - (SCALE / 256.0) * fin[0, 1]
    out = np.array(np.float32(total / N))
    if _trace:
        kernel._last_result = res
    return out


# revision 16
# speedup vs baseline: 1.3184x; 1.3184x over previous
"""NT-Xent loss on 8 Trainium2 NeuronCores.

Full inputs in, full (scalar) output out. Row-parallel sharding: core c
owns rows [1024c, 1024c+1024) of the 8192-row feature matrix and computes
its block of the similarity matrix against all columns. Inputs are
row-rotated per core so the single SPMD program sees its own rows at
local positions 0..1023 (static diagonal mask / positive-pair columns).

v2.1 (ACT-engine-bound problem: 8.4M exps/core at 1 elem/cyc/lane):
  features staged bf16 on host (halves DMA, enables DVE 2x modes)
  -> chunk 0 loaded in halves over both HWDGE rings and normalized per
     half (shortest time to first ACT work), chunks 1-3 pipelined behind
  -> pos-term multiplies hoisted before the main stream, its colsum and
     reduce tucked between groups 1 and 2
  -> uninterrupted main stream: PE bf16 row-block matmuls ping-ponging
     two 4-bank PSUM tiles while ACT runs back-to-back exp(x/T) with
     free row-sum accumulation
  -> tail is one [128,2] DMA; the host does the final cross-partition
     and cross-core sums.
"""
import numpy as np
import ml_dtypes

import concourse.bass as bass  # noqa: F401
import concourse.tile as tile
import concourse.bacc as bacc_mod
from concourse import bacc, mybir
from concourse.bass_utils import run_bass_kernel_spmd
from concourse.hw_specs import get_activation_tables as _real_tables

B, D = 4096, 256
N = 2 * B                # 8192 rows/cols of sim matrix
NCORES = 8
RPC = N // NCORES        # 1024 rows per core
TEMP = 0.07
SCALE = 1.0 / TEMP
KG = 2                   # contraction groups: D = 256 = 2 * 128
CH = 2048                # column chunk (psum tile width)
NCH = N // CH            # 4 chunks
MT = RPC // 128          # 8 M-tiles per core
NEG = -1.0e9

AF = mybir.ActivationFunctionType
ALU = mybir.AluOpType
AX = mybir.AxisListType
f32 = mybir.dt.float32
bf16 = mybir.dt.bfloat16

_CACHE = {}


def _pinned_tables(arch):
    """Keep Exp/Ln only in natural_log_exp_and_others so the act-table
    insertion pass picks one set for the whole kernel (no reload thrash)."""
    tables = _real_tables(arch)
    out = {}
    for name, funcs in tables.items():
        if name != "natural_log_exp_and_others":
            funcs = {f for f in funcs if f.name not in ("Exp", "Ln")}
        out[name] = funcs
    return out


def _build_nc():
    bacc_mod.get_activation_tables = _pinned_tables
    nc = bacc.Bacc("TRN2", target_bir_lowering=False, debug=False,
                   enable_asserts=False, num_devices=NCORES,
                   num_swdge_queues=2)

    ztb_d = nc.dram_tensor("ztb", [KG, 128, N], bf16, kind="ExternalInput")
    cones_d = nc.dram_tensor("cones", [128, 128], bf16, kind="ExternalInput")
    negid_d = nc.dram_tensor("negid", [128, 128], f32, kind="ExternalInput")
    out_d = nc.dram_tensor("out", [128, 2], f32, kind="ExternalOutput")

    with tile.TileContext(nc) as tc:
        with (
            tc.tile_pool(name="singles", bufs=1) as singles,
            tc.tile_pool(name="nfp", bufs=1) as nfp,
            tc.tile_pool(name="sqp", bufs=2) as sqp,
            tc.tile_pool(name="invp", bufs=2) as invp,
            tc.tile_pool(name="expp", bufs=2) as expp,
            tc.tile_pool(name="ps", bufs=2, space="PSUM") as ps,
        ):
            # constants ride the SWDGE ring so feature loads own the HWDGEs
            cones = singles.tile([128, 128], bf16, tag="cones")
            nc.gpsimd.dma_start(out=cones, in_=cones_d.ap())
            negid = singles.tile([128, 128], f32, tag="negid")
            nc.gpsimd.dma_start(out=negid, in_=negid_d.ap())

            # lse exp-sum accumulators: column m*NCH+g
            sums = singles.tile([128, MT * NCH], f32, tag="sums")

            ztb = [nfp.tile([128, KG, CH], bf16, tag=f"ztb{g}",
                            name=f"ztb{g}") for g in range(NCH)]
            nf = [nfp.tile([128, KG, CH], bf16, tag=f"nf{g}",
                           name=f"nf{g}") for g in range(NCH)]

            ztb_ap = ztb_d.ap()

            def load(g, col0, width, eng):
                eng.dma_start(
                    out=ztb[g][:, :, col0:col0 + width],
                    in_=ztb_ap[:, :, CH * g + col0:CH * g + col0 + width]
                    .rearrange("k p c -> p k c"))

            load(0, 0, 1024, nc.sync)       # chunk 0 halved over both
            load(0, 1024, 1024, nc.scalar)  # HWDGE rings: earliest start
            load(1, 0, CH, nc.scalar)
            load(2, 0, CH, nc.gpsimd)
            load(3, 0, CH, nc.sync)

            def normalize(g, col0, width):
                """ztb[g] cols [col0,col0+width) -> nf[g] = z / ||col||."""
                zs = ztb[g][:, :, col0:col0 + width]
                sq_t = sqp.tile([128, KG, width], bf16, tag="sq",
                                name=f"sq{g}_{col0}")
                nc.vector.tensor_mul(sq_t, zs, zs)
                nn_ps = ps.tile([128, width], f32, tag="ps",
                                name=f"nn{g}_{col0}")
                for n in range(width // 512):
                    for kg in range(KG):
                        nc.tensor.matmul(
                            nn_ps[:, 512 * n:512 * (n + 1)], cones,
                            sq_t[:, kg, 512 * n:512 * (n + 1)],
                            start=(kg == 0), stop=(kg == KG - 1))
                # 1/sqrt(nn) == exp(-0.5 * ln(nn))
                lnv = invp.tile([128, width], f32, tag="lnv",
                                name=f"lnv{g}_{col0}")
                nc.scalar.activation(lnv, nn_ps, AF.Ln)
                inv = invp.tile([128, width], bf16, tag="inv",
                                name=f"inv{g}_{col0}")
                nc.scalar.activation(inv, lnv, AF.Exp, scale=-0.5)
                for kg in range(KG):
                    nc.vector.tensor_mul(nf[g][:, kg, col0:col0 + width],
                                         zs[:, kg, :], inv)

            def main_group(g):
                for m in range(MT):
                    sim_ps = ps.tile([128, CH], f32, tag="ps", name="sim_ps")
                    for kg in range(KG):
                        lhsT = nf[0][:, kg, 128 * m:128 * (m + 1)]
                        for n in range(CH // 512):
                            nc.tensor.matmul(
                                sim_ps[:, 512 * n:512 * (n + 1)], lhsT,
                                nf[g][:, kg, 512 * n:512 * (n + 1)],
                                start=(kg == 0), stop=(kg == KG - 1),
                                skip_group_check=True)
                    if g == 0:
                        # mask self-similarity: row block m's diagonal is at
                        # columns [128m, 128m+128) of group 0
                        sl = sim_ps[:, 128 * m:128 * (m + 1)]
                        nc.vector.tensor_add(sl, sl, negid)
                    exp_sc = expp.tile([128, CH], bf16, tag="exp",
                                       name="exp_sc")
                    idx = m * NCH + g
                    nc.scalar.activation(exp_sc, sim_ps, AF.Exp, scale=SCALE,
                                         accum_out=sums[:, idx:idx + 1])

            normalize(0, 0, 1024)
            normalize(0, 1024, 1024)
            for g in range(1, NCH):
                normalize(g, 0, CH)

            # positive term inputs: partner of local row i is local column
            # i of group 2. pos_i = <nf_i, nf_{i+4096}>.  The multiplies run
            # before the main stream; colsum+reduce slot in between groups.
            fin = singles.tile([128, 2], f32, tag="fin")
            tmp_pos = sqp.tile([128, KG, RPC], bf16, tag="tpos")
            for kg in range(KG):
                nc.vector.tensor_mul(tmp_pos[:, kg, :],
                                     nf[0][:, kg, 0:RPC],
                                     nf[2][:, kg, 0:RPC])

            main_group(0)
            main_group(1)

            pos_ps = ps.tile([128, RPC], f32, tag="ps")
            for n in range(RPC // 512):
                for kg in range(KG):
                    nc.tensor.matmul(
                        pos_ps[:, 512 * n:512 * (n + 1)], cones,
                        tmp_pos[:, kg, 512 * n:512 * (n + 1)],
                        start=(kg == 0), stop=(kg == KG - 1))
            # pos_ps rows are identical (colsum replicated); reduce row-wise
            nc.vector.tensor_reduce(fin[:, 1:2], pos_ps, axis=AX.X,
                                    op=ALU.add)

            main_group(2)
            main_group(3)

            # lse per row: ln(sum over the NCH group sums)
            rowsum = singles.tile([128, MT], f32, tag="rowsum")
            nc.vector.tensor_reduce(
                rowsum, sums.rearrange("p (m g) -> p m g", g=NCH),
                axis=AX.X, op=ALU.add)
            lse8 = singles.tile([128, MT], f32, tag="lse8")
            nc.scalar.activation(lse8, rowsum, AF.Ln)
            nc.vector.tensor_reduce(fin[:, 0:1], lse8, axis=AX.X, op=ALU.add)
            # host combines: sum_p fin[p,0] - SCALE * fin[0,1]
            nc.sync.dma_start(out=out_d.ap(), in_=fin)

    nc.compile()
    return nc


def _get_nc():
    if "nc" not in _CACHE:
        _CACHE["nc"] = _build_nc()
    return _CACHE["nc"]


def _in_maps(z_i, z_j):
    feats = np.concatenate([np.asarray(z_i, dtype=np.float32),
                            np.asarray(z_j, dtype=np.float32)], axis=0)
    cones = np.ones((128, 128), dtype=ml_dtypes.bfloat16)
    negid = (NEG * np.eye(128)).astype(np.float32)
    maps = []
    for c in range(NCORES):
        zr = np.roll(feats, -RPC * c, axis=0)          # [N, D]
        ztb = np.ascontiguousarray(zr.T).reshape(KG, 128, N).astype(
            ml_dtypes.bfloat16)
        maps.append({"ztb": ztb, "cones": cones, "negid": negid})
    return maps


def kernel(z_i, z_j, _trace=False, _trace_kwargs=None):
    nc = _get_nc()
    maps = _in_maps(z_i, z_j)
    res = run_bass_kernel_spmd(nc, maps, core_ids=list(range(NCORES)),
                               trace=_trace, **(_trace_kwargs or {}))
    total = 0.0
    for c in range(NCORES):
        fin = np.asarray(res.results[c]["out"], dtype=np.float64)
        total += fin[:, 0].sum() - SCALE * fin[0, 1]
    out = np.array(np.float32(total / N))
    if _trace:
        kernel._last_result = res
    return out
